# revision 65
# baseline (speedup 1.0000x reference)
"""Trainium2 Bass kernel for DiagonalKernelAverageV2.

Math: for each (b, ch) image X [512, 512] and each of 4 corners, the output
at index i is the mean over the L-shaped shell of the i-th nested corner
square:  shell[i] = d[i] - d[i-1],  d[i] = sum of the (i+1)x(i+1) corner
window,  counts[i] = 2i+1.

Only two shell families are computed directly (top-left and top-right); the
bottom corners follow from row/col totals:
    shell_tl[i] = sum_{c<=i} X[i,c] + sum_{r<i}  X[r,i]
    shell_tr[i] = sum_{c>=511-i} X[i,c] + sum_{r<i} X[r,511-i]
    shell_br[i] = S[511-i] + ST[511-i] - shell_tl[511-i]
    shell_bl[i] = S[511-i] + ST[i]     - shell_tr[511-i]
(S = row sums, ST = col sums.)

Per-core layout: batch-sharded (4 batches x 8 channels per core).  Each image
is 4 row-tiles [128, 512].

v3 engine plan (cost-model balanced):
  - Input loads: the f32 DRAM is bitcast to bf16 and only the high halves
    ([..., 1::2]) are loaded via plain HWDGE dma_start from SP/Act -- DMA cost
    is metered on OUTPUT bytes, so this halves DMA-engine time vs f32.  The
    bf16 truncation bias (~x*(1-1/512·ln2... measured on randn) is corrected
    by scaling the output weights.
  - Pool: the masked stt ops; output weighting; part of the assembly.
  - DVE: block row-sums via three fp16 tensor_tensor folds (2x perf mode) +
    one small grouped reduce; part of the final assembly.
  - TensorE: all column-side quantities via tiny matmuls with X (or the
    masked products) as the 128x128 STATIONARY operand and constant 0/1
    [128, 10] moving weights; results land per-column on PSUM partitions in
    exactly the TQ layout (reversed variants via free-reversed stationary),
    so no transposes and no wide PSUM staging are needed.
  - ScalarE: one small PSUM->SBUF copy per image.
Bottom-corner outputs are written in source order and flipped on the host.
"""

import numpy as np

SIZE = 512
NT = 4  # row tiles per image
NCH = 8  # channels per batch
NB_CORE = 4  # batches per core
N_CORES = 8
NQ = 6  # per-column quantity slots
GCHUNK = 4  # images per input DMA
N_STT_POOL = 8  # how many of the 8 masked stt ops go to the Pool engine
REV_STATIONARY = True  # reversed-AP stationary for the reversed transposes
DBG_STAGE = 2  # debug aid: 1 = per-image pipeline only, 2 = full kernel


def build_nc():
    import concourse.bass as bass
    import concourse.bacc as bacc
    import concourse.mybir as mybir
    from concourse.tile import TileContext

    f32 = mybir.dt.float32
    f16 = mybir.dt.bfloat16
    fh16 = mybir.dt.float16
    nc = bacc.Bacc()

    x = nc.dram_tensor("x", [NB_CORE, NCH, SIZE, SIZE], f32, kind="ExternalInput")
    msu_d = nc.dram_tensor("msu", [128, 2, 128], f32, kind="ExternalInput")
    wq_d = nc.dram_tensor("wq", [128, NT + NT + 2, NQ], f32, kind="ExternalInput")
    wqh_d = nc.dram_tensor("wqh", [128, NT, NQ], f16, kind="ExternalInput")
    wg_d = nc.dram_tensor("wg", [128, NCH, NT], f32, kind="ExternalInput")
    wrevg_d = nc.dram_tensor("wrevg", [128, NCH, NT], f32, kind="ExternalInput")
    out = nc.dram_tensor("out", [NB_CORE, SIZE, 4 * NCH], f32, kind="ExternalOutput")
    out2 = nc.dram_tensor(
        "out2", [NB_CORE, 128, NCH * NT * NQ], f32, kind="ExternalOutput"
    )
    if DBG_STAGE < 2:
        dbg_tq = nc.dram_tensor(
            "dbg_tq", [NB_CORE, 128, NCH * NT * NQ], f32, kind="ExternalOutput"
        )
        dbg_b = nc.dram_tensor(
            "dbg_b", [NB_CORE, 128, NCH * NT * NT], f32, kind="ExternalOutput"
        )

    ADD = mybir.AluOpType.add
    MULT = mybir.AluOpType.mult
    SUB = mybir.AluOpType.subtract
    AX = mybir.AxisListType.X

    with TileContext(nc) as tc:
        with (
            tc.tile_pool(name="consts", bufs=1) as consts,
            tc.tile_pool(name="xs", bufs=1) as xpool,
            tc.tile_pool(name="folds", bufs=2) as fpool,
            tc.tile_pool(name="pp", bufs=2) as ppool,
            tc.tile_pool(name="perb", bufs=2) as bpool,
            tc.tile_pool(name="small", bufs=2) as spool,
            tc.tile_pool(name="pst", bufs=3, space="PSUM") as pst,
        ):
            msu = consts.tile([128, 2, 128], f32)
            wq = consts.tile([128, NT + NT + 2, NQ], f32)
            wqh = consts.tile([128, NT, NQ], f16)
            wg = consts.tile([128, NCH, NT], f32)
            wrevg = consts.tile([128, NCH, NT], f32)

            def emit_consts():
                nc.sync.dma_start(out=msu, in_=msu_d[:])
                nc.sync.dma_start(out=wq, in_=wq_d[:])
                nc.sync.dma_start(out=wqh, in_=wqh_d[:])
                nc.sync.dma_start(out=wg, in_=wg_d[:])
                nc.sync.dma_start(out=wrevg, in_=wrevg_d[:])

            from concourse.bass import _add_dep_helper

            # Input loads, balanced across three DMA-issuing engine streams
            # (each engine's instruction stream serializes with its DMA
            # transfer time in the cost model):
            #   g0-2 -> SP,  g3-5 -> Act: bf16 HIGH-HALF loads (bitcast +
            #     stride-2 slice of the f32 data -> half the DMA bytes), one
            #     [128, 512] slab per (image, row-tile).
            #   g6-7 -> Pool: SWDGE f32->bf16 casting load, one 2-image chunk.
            # Every image has a dedicated buffer (no waits), so loads
            # prefetch as deep as their position in the engine stream allows.
            N_TRUNC = 6  # channels loaded as f32 (rest Pool-cast to bf16)
            xslab = {}
            xcast = {}
            ximg = {}

            def emit_img_load(b, g, eng):
                eng.dma_start(
                    out=xslab[b][:, g],
                    in_=x[b, g].rearrange("(t p) c -> p t c", p=128),
                )

            def emit_cast_load(b):
                Xc = xpool.tile(
                    [128, 2, NT, SIZE], f16, tag=f"xc{b}", name=f"Xc_{b}"
                )
                nc.gpsimd.dma_start(
                    out=Xc.rearrange("p g t c -> p (g t) c"),
                    in_=x[b, N_TRUNC:].rearrange(
                        "g (t p) c -> p (g t) c", p=128
                    ),
                )
                xcast[b] = Xc
                ximg[(b, N_TRUNC)] = Xc[:, 0]
                ximg[(b, N_TRUNC + 1)] = Xc[:, 1]

            for b in range(NB_CORE):
                X6 = xpool.tile(
                    [128, N_TRUNC, NT, SIZE],
                    f32,
                    tag=f"x{b % 2}",
                    name=f"X6_{b}",
                )
                xslab[b] = X6
                for g in range(N_TRUNC):
                    ximg[(b, g)] = X6[:, g]
            # upfront: batch-0 and batch-1 loads (g0-2 -> SP, g3-5 -> Act)
            for b in range(2):
                for g in range(3):
                    emit_img_load(b, g, nc.sync)
                if b == 0:
                    emit_consts()
                for g in range(3, N_TRUNC):
                    emit_img_load(b, g, nc.scalar)
                emit_cast_load(b)
            # batches 2-3 are paced inside the image loop below
            act_pending = [
                (b, g) for b in range(2, NB_CORE) for g in range(N_TRUNC)
            ]
            act_emitted = 0

            prev_pe_last = None
            for b in range(NB_CORE):
                B_G = bpool.tile([128, NCH, NT, NT], f32, tag="bg")
                RSsu = bpool.tile([128, NCH, NT], f32, tag="rssu")
                RS2su = bpool.tile([128, NCH, NT], f32, tag="rs2su")
                TQ = pst.tile(
                    [128, NCH, NT, NQ], f32, tag="tq", name=f"TQp_{b}", bufs=2
                )
                if b + 1 >= 2 and b + 1 < NB_CORE:
                    emit_cast_load(b + 1)

                # block row sums via three fp16 folds (2x DVE mode) + one
                # grouped reduce.  n images are folded in a single op chain
                # (batch 0 folds per-image to pipeline with the loads).
                def fold_chain(src_ap, n, bg_out, tag=0):
                    # src_ap: [128, n*NT*4, 128] bf16 view
                    F1 = fpool.tile(
                        [128, n * NT * 4, 64], fh16, tag=f"f1_{n}_{tag}", bufs=1
                    )
                    nc.vector.tensor_tensor(
                        F1, src_ap[:, :, 0:64], src_ap[:, :, 64:128], op=ADD
                    )
                    F2 = fpool.tile(
                        [128, n * NT * 4, 32], fh16, tag=f"f2_{n}_{tag}", bufs=1
                    )
                    nc.vector.tensor_tensor(
                        F2, F1[:, :, 0:32], F1[:, :, 32:64], op=ADD
                    )
                    F3 = fpool.tile(
                        [128, n * NT * 4, 16], fh16, tag=f"f3_{n}_{tag}", bufs=1
                    )
                    nc.vector.tensor_tensor(
                        F3, F2[:, :, 0:16], F2[:, :, 16:32], op=ADD
                    )
                    F4 = fpool.tile(
                        [128, n * NT * 4, 8], fh16, tag=f"f4_{n}_{tag}", bufs=1
                    )
                    nc.vector.tensor_tensor(
                        F4, F3[:, :, 0:8], F3[:, :, 8:16], op=ADD
                    )
                    nc.vector.tensor_reduce(
                        out=bg_out, in_=F4, axis=AX, op=ADD
                    )

                fold_chain(
                    xcast[b].rearrange("p g t (j c) -> p (g t j) c", c=128),
                    2,
                    B_G[:, N_TRUNC:].rearrange("p g t j -> p (g t j)"),
                )
                if b > 1:
                    fold_chain(
                        xslab[b][:, 3:6].rearrange(
                            "p g t (j c) -> p (g t j) c", c=128
                        ),
                        3,
                        B_G[:, 3:6].rearrange("p g t j -> p (g t j)"),
                        tag=1,
                    )
                    fold_chain(
                        xslab[b][:, 0:3].rearrange(
                            "p g t (j c) -> p (g t j) c", c=128
                        ),
                        3,
                        B_G[:, 0:3].rearrange("p g t j -> p (g t j)"),
                    )

                # consume images in load-readiness order (batch 0 leads
                # with the specially-split image (0,0); later batches lead
                # with the early-landing cast/Act images)
                g_order = (
                    (0, 3, 6, 1, 4, 7, 2, 5)
                    if b <= 1
                    else (6, 7, 3, 4, 5, 0, 1, 2)
                )
                for g in g_order:
                    X = ximg[(b, g)]  # [128, NT, SIZE] bf16
                    if b <= 1 and g < N_TRUNC:
                        fold_chain(
                            X.rearrange("p t (j c) -> p (t j) c", c=128),
                            1,
                            B_G[:, g].rearrange("p t j -> p (t j)"),
                        )
                    # masked products + fused row sums (strict-upper mask):
                    # out = (block * 1.0) * msu, accum_out = rowsum(out)
                    # split across DVE and Pool to balance engine load
                    PP = ppool.tile([128, 2, SIZE], f32)
                    stt_ops = []
                    for t in range(NT):
                        stt_ops.append(
                            dict(
                                out=PP[:, 0, 128 * t : 128 * (t + 1)],
                                in0=X[:, t, 128 * t : 128 * (t + 1)],
                                accum_out=RSsu[:, g, t : t + 1],
                                mask=0,
                            )
                        )
                        stt_ops.append(
                            dict(
                                out=PP[:, 1, 128 * t : 128 * (t + 1)],
                                in0=X[:, t, 128 * (3 - t) : 128 * (4 - t)],
                                accum_out=RS2su[:, g, t : t + 1],
                                mask=1,
                            )
                        )
                    for oi, kw in enumerate(stt_ops):
                        eng = nc.vector
                        eng.scalar_tensor_tensor(
                            out=kw["out"],
                            in0=kw["in0"],
                            scalar=1.0,
                            in1=msu[:, kw["mask"]],
                            op0=MULT,
                            op1=MULT,
                            accum_out=kw["accum_out"],
                        )
                    # column-side quantities via tiny matmuls: X (or PP) block
                    # as the 128x128 STATIONARY, constant 0/1 [128, NQ] moving.
                    # psumT[k, s, q] = quantity q at column 128s+k:
                    #   q 0-2: CPfx[1..3], 3: ST, 4: colsum(P1), 5: colsum(P2rev)
                    #   q 6-9: same as 0-3 but at column 128s+(127-k) (reversed)
                    for s in range(NT):
                        ops = []
                        for t in range(NT):
                            ops.append(
                                dict(lhsT=X[:, t, 128 * s : 128 * (s + 1)], w=t)
                            )
                        ops.append(
                            dict(lhsT=PP[:, 0, 128 * s : 128 * (s + 1)], w=2 * NT)
                        )
                        ops.append(
                            dict(
                                lhsT=PP[:, 1, 128 * s : 128 * (s + 1)], w=2 * NT + 1
                            )
                        )
                        isf32 = g < N_TRUNC
                        for oi, op in enumerate(ops):
                            if op["w"] >= 2 * NT or isf32:
                                rhs_w = wq[:, op["w"], :]  # f32 (PP is f32)
                            else:
                                rhs_w = wqh[:, op["w"], :]  # bf16 X images
                            mm = nc.tensor.matmul(
                                TQ[:, g, s, :],
                                lhsT=op["lhsT"],
                                rhs=rhs_w,
                                start=(oi == 0),
                                stop=(oi == len(ops) - 1),
                            )
                            # keep strict PE program order so accumulation
                            # groups never interleave
                            if prev_pe_last is not None:
                                _add_dep_helper(
                                    mm.ins, prev_pe_last.ins, sync=False,
                                    reason="PE group ordering",
                                )
                            prev_pe_last = mm
                    # pace batches 2-3's loads: one per image, alternating
                    # engines, so batch b+1 is resident before its folds
                    img_idx = b * NCH + g
                    target = min(len(act_pending), max(0, img_idx - 6))
                    while act_emitted < target:
                        pb, pg = act_pending[act_emitted]
                        emit_img_load(
                            pb, pg, nc.sync if pg < 3 else nc.scalar
                        )
                        act_emitted += 1

                # stage the quantities to SBUF and ship them to the host;
                # the host performs the partition-reversed combines for the
                # tr/bl corners (engines cannot reverse partitions, and the
                # HW compiler rejects negative-stride DMA partition steps)
                TQs = bpool.tile([128, NCH * NT * NQ], f32, tag="tqs")
                nc.scalar.copy(TQs, TQ.rearrange("p g t q -> p (g t q)"))
                reng = nc.sync if b % 2 == 0 else nc.scalar
                reng.dma_start(out=out2[b], in_=TQs)

                if DBG_STAGE == 1:
                    nc.sync.dma_start(
                        out=dbg_tq[b], in_=TQ.rearrange("p a b c -> p (a b c)")
                    )
                    nc.sync.dma_start(
                        out=dbg_b[b], in_=B_G.rearrange("p a b c -> p (a b c)")
                    )
                    continue
                # ---- per-batch assembly (all [128, (g), (t)] strided ops) ----
                def bg_ap(base, tstep):
                    return bass.AP(
                        tensor=B_G.tensor,
                        offset=B_G[:, 0, 0, 0:1].offset + base,
                        ap=[B_G[:, 0, 0, 0:1].ap[0]] + [[16, NCH], [tstep, NT]],
                    )

                def tq_ap(base, tstep, nt=NT):
                    return bass.AP(
                        tensor=TQs.tensor,
                        offset=TQs[:, 0:1].offset + base,
                        ap=[TQs[:, 0:1].ap[0]] + [[NT * NQ, NCH], [tstep, nt]],
                    )


                PI = bpool.tile([128, NCH, 5, NT], f32, tag="pi")

                def pi_ap(base, tstep, nt=NT):
                    return bass.AP(
                        tensor=PI.tensor,
                        offset=PI[:, 0, 0, 0:1].offset + base,
                        ap=[PI[:, 0, 0, 0:1].ap[0]] + [[20, NCH], [tstep, nt]],
                    )

                nc.vector.memset(PI[:, :, 0, :], 0.0)
                nc.vector.tensor_copy(PI[:, :, 1, :], B_G[:, :, :, 0])
                for m in range(2, 5):
                    nc.vector.tensor_tensor(
                        PI[:, :, m, :], PI[:, :, m - 1, :], B_G[:, :, :, m - 1],
                        op=ADD,
                    )

                sh_tl = spool.tile([128, NCH, NT], f32, tag="shtl")
                sh_tr = spool.tile([128, NCH, NT], f32, tag="shtr")
                O = spool.tile([128, NT, 4 * NCH], f32, tag="obuf")
                # shell_tl = B[t][t] - RSsu + PI[m=t] + CPfx[m=t] + CS1
                nc.vector.tensor_tensor(sh_tl, bg_ap(0, 5), RSsu, op=SUB)
                nc.vector.tensor_tensor(sh_tl, sh_tl, pi_ap(0, 5), op=ADD)
                nc.vector.tensor_tensor(
                    sh_tl[:, :, 1:4], sh_tl[:, :, 1:4], tq_ap(NQ, NQ + 1, 3), op=ADD
                )
                nc.vector.tensor_tensor(sh_tl, sh_tl, tq_ap(4, NQ), op=ADD)
                # A_r = B[t][3-t] - RS2su + S - PI[m=4-t]  (tr row part;
                # the column part is combined on the host from out2)
                nc.vector.tensor_tensor(sh_tr, bg_ap(3, 3), RS2su, op=SUB)
                nc.vector.tensor_tensor(sh_tr, sh_tr, pi_ap(16, 1), op=ADD)
                nc.vector.tensor_tensor(
                    O[:, :, 2 * NCH : 3 * NCH].rearrange("p t g -> p g t"),
                    sh_tr, pi_ap(16, -3), op=SUB,
                )
                # S plane (host needs it for bl)
                nc.vector.tensor_copy(
                    O[:, :, 3 * NCH :].rearrange("p t g -> p g t"), pi_ap(16, 1)
                )

                # br (src order): u = ST - shell_tl + S
                u = spool.tile([128, NCH, NT], f32, tag="u")
                nc.vector.tensor_tensor(u, tq_ap(3, NQ), sh_tl, op=SUB)
                nc.vector.tensor_tensor(u, u, pi_ap(16, 1), op=ADD)
                for ci, (src, wt) in enumerate([(sh_tl, wg), (u, wrevg)]):
                    nc.vector.tensor_tensor(
                        O[:, :, ci * NCH : (ci + 1) * NCH],
                        src.rearrange("p g t -> p t g"),
                        wt.rearrange("p g t -> p t g"),
                        op=MULT,
                    )
                nc.sync.dma_start(
                    out=out[b].rearrange("(t p) c -> p t c", p=128), in_=O
                )
    nc.compile()
    return nc


_CORR = None


def _g_corr():
    """Per-channel scale: bf16-truncation bias correction for the
    truncation-loaded channels 0-5; RN-cast channels 6-7 need none."""
    global _CORR
    if _CORR is None:
        rng = np.random.default_rng(1234)
        s = rng.standard_normal(1 << 20).astype(np.float32)
        s_tr = (s.view(np.uint32) & np.uint32(0xFFFF0000)).view(np.float32)
        corr = float((s * s_tr).sum() / (s_tr * s_tr).sum())
        del corr  # f32 loads are exact; bf16-cast channels are RN (unbiased)
        _CORR = np.ones(NCH)
    return _CORR


def make_consts():
    import ml_dtypes

    bf16 = ml_dtypes.bfloat16

    r = np.arange(128)
    msu0 = (r[None, :] > r[:, None]).astype(np.float32)  # [c > r] strict
    msu1 = msu0[:, ::-1].copy()  # [c < 127 - r] anti mask
    msu = np.stack([msu0, msu1], axis=1)  # [128, 2, 128]
    # moving weights for the stationary-X matmuls: [128, 10 slots, NQ]
    # slot t in 0..3: direct X tile t -> q 0-2 CPfx[1..3] ([t<m]), q 3 ST
    # slot 8: PP0 -> q 4 (CS1);  slot 9: PP1 -> q 5 (CS2fwd)
    wq = np.zeros((128, NT + NT + 2, NQ), np.float32)
    for t in range(NT):
        for m in range(1, 4):
            if t < m:
                wq[:, t, m - 1] = 1.0
        wq[:, t, 3] = 1.0
    wq[:, 2 * NT, 4] = 1.0
    wq[:, 2 * NT + 1, 5] = 1.0
    wqh = wq[:, 0:NT].astype(bf16)
    i_pt = (r[:, None] + 128 * np.arange(NT)[None, :]).astype(np.float64)
    w_pt = (1.0 / (2 * i_pt + 1)).astype(np.float64)  # [128, NT]
    wrev_pt = (1.0 / (1023.0 - 2 * i_pt)).astype(np.float64)
    # channels 0..5 are loaded via bf16 truncation -> bias-corrected;
    # channels 6..7 are round-to-nearest cast -> no correction
    gcorr = _g_corr()[None, :, None]
    wg = (np.tile(w_pt[:, None, :], (1, NCH, 1)) * gcorr).astype(np.float32)
    wrevg = (np.tile(wrev_pt[:, None, :], (1, NCH, 1)) * gcorr).astype(
        np.float32
    )
    return dict(msu=msu, wq=wq, wqh=wqh, wg=wg, wrevg=wrevg)


_NC = None


def _get_nc():
    global _NC
    if _NC is None:
        _NC = build_nc()
    return _NC


def kernel(x: np.ndarray) -> np.ndarray:
    from concourse.bass_utils import run_bass_kernel_spmd

    x = np.asarray(x, dtype=np.float32)
    B = x.shape[0]
    consts = make_consts()
    per_core = B // N_CORES
    assert per_core == NB_CORE
    in_maps = [
        {"x": x[c * per_core : (c + 1) * per_core], **consts}
        for c in range(N_CORES)
    ]
    nc = _get_nc()
    res = run_bass_kernel_spmd(nc, in_maps, core_ids=list(range(N_CORES)))
    outs = []
    for r in res.results:
        outs.append(_host_combine(r["out"], r["out2"]))
    return np.concatenate(outs, axis=0)


def _host_combine(o, tq):
    """o: [NB, 512, 32] = [tl*w | u*wrev | A_r | S]; tq: [NB, 128, 192].

    Rebuilds the tr/bl corners: their column-side terms live at partition
    127-p of the quantities tile, which the hardware cannot re-index, so the
    flip happens here."""
    import ml_dtypes  # noqa: F401

    nb = o.shape[0]
    tqr = tq.reshape(nb, 128, NCH, NT, NQ)
    A_r = o[:, :, 2 * NCH : 3 * NCH]
    S = o[:, :, 3 * NCH :]
    C1 = np.zeros((nb, SIZE, NCH), np.float32)
    for sblk in range(3):
        C1[:, 128 * sblk : 128 * (sblk + 1), :] = tqr[:, :, :, sblk, 2 - sblk]
    C2 = tqr[:, :, :, :, 5].transpose(0, 3, 1, 2).reshape(nb, SIZE, NCH)
    C3 = tqr[:, :, :, :, 3].transpose(0, 3, 1, 2).reshape(nb, SIZE, NCH)
    i = np.arange(SIZE)
    tf = (i // 128) * 128 + 127 - (i % 128)
    corr_g = _g_corr()[None, None, :]
    w = (corr_g / (2 * i + 1)[None, :, None]).astype(np.float64)
    wrev = (corr_g / (1023 - 2 * i)[None, :, None]).astype(np.float64)
    tr_raw = A_r + C1[:, 511 - i, :] + C2[:, tf, :]
    tr_out = (tr_raw * w).astype(np.float32)
    bl_src = (C3[:, 511 - i, :] + S - tr_raw) * wrev
    bl_out = bl_src[:, ::-1, :].astype(np.float32)
    br_out = o[:, ::-1, NCH : 2 * NCH]
    return np.concatenate(
        [o[:, :, 0:NCH], tr_out, bl_out, br_out], axis=2
    ).astype(np.float32)
